# revision 1
# baseline (speedup 1.0000x reference)
"""Trainium2 Bass kernel for nn_NeRFMLPNetwork (StyleGAN-style modulated 1x1-conv MLP).

Network (per layer): s = affine(w_lat); y = conv1x1(x * s); y = y * rsqrt(demod) + b;
out = lrelu(y) * sqrt(2).  8 layers (60->128, then 7x 128->128), B=4, H*W=32768.

Strategy:
  - Data parallel over H*W: each of 8 cores handles 4096 spatial points (all batches).
  - Per (layer, batch) fold modulation s into the weight: Wmod[c,o] = convT[c,o]*s[b,c],
    kept in f32r (full-rate matmul, ~2^-13 precision).  Demod scale d and bias are
    applied in the epilogue: out = prelu(psum*dscale + sqrt2*cb, alpha=0.2), where
    dscale = sqrt(2/(v+eps)) folds in the sqrt(2) lrelu gain.
  - Epilogue split between ScalarE (Prelu activation) and VectorE (custom DVE op
    SCALE_BIAS_LRELU: out = max(z, 0.2z), z = in*s0+s1) so neither engine bottlenecks.
  - Style path (tiny) on device: s via PE matmul over 4 K-chunks, demod sum via PE
    matmul of squared weights, sqrt on ScalarE, reciprocal on VectorE.

Host-side prep is layout only: transposes/reshapes of the small parameter tensors
plus folding the constant sqrt(2) into the conv bias.
"""

import numpy as np

import concourse.bacc as bacc
import concourse.mybir as mybir
import concourse.tile as tile
from concourse.bass_utils import run_bass_kernel_spmd

# ---------------------------------------------------------------------------
# Custom DVE op: out = max(z, z*imm2) with z = in0*s0 + s1   (leaky relu)
# ---------------------------------------------------------------------------
import concourse.dve_ops as dve_ops_mod
from concourse.dve_spec import Spec, Src0, C0, C1, C2, maxx, lower as _dve_lower
from concourse.dve_spec import _has_src1
from concourse.dve_uop import DveOpSpec


def _sbl_ref(in0, in1, s0, s1, imm2):
    z = in0.astype(np.float32) * s0 + s1
    return np.maximum(z, z * imm2)


_z = Src0 * C0 + C1
_SBL_SPEC = Spec(body=maxx(_z, _z * C2), reference=_sbl_ref)
SCALE_BIAS_LRELU = dve_ops_mod.DveOp(
    "SCALE_BIAS_LRELU", _SBL_SPEC, subdim=False, uops_sha={}
)
if "SCALE_BIAS_LRELU" not in dve_ops_mod._SUB_OPCODE_FOR_NAME:
    dve_ops_mod.OPS.append(SCALE_BIAS_LRELU)
    dve_ops_mod.CUSTOM_DVE_SPECS["SCALE_BIAS_LRELU"] = _SBL_SPEC
    dve_ops_mod._SUB_OPCODE_FOR_NAME["SCALE_BIAS_LRELU"] = (
        max(dve_ops_mod._SUB_OPCODE_FOR_NAME.values()) + 1
    )
for _ver in ("v3", "v4"):
    _s = DveOpSpec(
        name="SCALE_BIAS_LRELU",
        opcode=dve_ops_mod.get_dve_sub_opcode("SCALE_BIAS_LRELU"),
        uops=_dve_lower(_SBL_SPEC, ver=_ver),
        rd1_en=_has_src1(_SBL_SPEC),
    )
    SCALE_BIAS_LRELU.uops_sha[_ver] = _s.sha(_ver)

# ---------------------------------------------------------------------------
# Problem constants (hardcoded per spec)
# ---------------------------------------------------------------------------
B, CIN, H, W, HID, WDIM, NB = 4, 60, 64, 512, 128, 512, 8
HWTOT = H * W                    # 32768
N_CORES = 8
SHARD = HWTOT // N_CORES         # 4096 spatial points per core
INV_SQRT_WDIM = float(1.0 / np.sqrt(WDIM))
SQRT2 = float(np.sqrt(2.0))
EPS = 1e-8

F32 = mybir.dt.float32
F32R = mybir.dt.float32r

GROUP = 1024                     # psum group columns (2 banks)
BLKCOLS = 4096                   # columns per processing block
SPLIT = 512                      # epilogue cols on ScalarE (bank-aligned); rest VectorE
NT = GROUP // 512                # matmuls per psum group
EPI_MODE = "split"               # 'split'(group-alternating) | 'splitcol' | 'act' | 'dve' | 'none'
ACT_SHARE = 17                   # of ACT_DEN groups go to ScalarE (rest VectorE)
ACT_DEN = 32

_COMPILED = None


def _build(K=1):
    """Build the program; K>1 unrolls the whole pipeline K times (for timing)."""
    nc = bacc.Bacc("TRN2", target_bir_lowering=False, debug=False,
                   num_devices=N_CORES)

    # x is declared f32r: raw f32 bits DMA directly; the PE rounds on read
    # (verified bit-identical to a DVE f32->f32r rounding copy).
    x_d = nc.dram_tensor("x", [B, CIN, SHARD], F32R, kind="ExternalInput").ap()
    wpT_d = nc.dram_tensor("wpT", [128, 4, NB, B], F32, kind="ExternalInput").ap()
    affT0_d = nc.dram_tensor("affT0", [128, 4, CIN], F32, kind="ExternalInput").ap()
    affTr_d = nc.dram_tensor("affTr", [128, 4, NB - 1, HID], F32, kind="ExternalInput").ap()
    ab0_d = nc.dram_tensor("ab0", [CIN, 1], F32, kind="ExternalInput").ap()
    abr_d = nc.dram_tensor("abr", [HID, NB - 1], F32, kind="ExternalInput").ap()
    cT0_d = nc.dram_tensor("cT0", [CIN, HID], F32, kind="ExternalInput").ap()
    cTr_d = nc.dram_tensor("cTr", [HID, NB - 1, HID], F32, kind="ExternalInput").ap()
    gcb0_d = nc.dram_tensor("gcb0", [HID, 1], F32, kind="ExternalInput").ap()
    gcbr_d = nc.dram_tensor("gcbr", [HID, NB - 1], F32, kind="ExternalInput").ap()
    y_d = nc.dram_tensor("y", [B, HID, SHARD], F32, kind="ExternalOutput").ap()

    COLS = B * SHARD             # 16384 columns resident per core

    with tile.TileContext(nc) as tc:
        with (
            tc.tile_pool(name="big", bufs=4) as big,
            tc.tile_pool(name="wts", bufs=1) as wts,
            tc.tile_pool(name="xst", bufs=4) as xst,
            tc.tile_pool(name="sty", bufs=3) as sty,
            tc.tile_pool(name="wmod", bufs=NB + 1) as wmodp,
            tc.tile_pool(name="dsc", bufs=NB + 1) as dscp,
            tc.tile_pool(name="ps", bufs=4, space="PSUM") as ps,
        ):
            # ---- weights DMA (small tensors first so style(0) unblocks early;
            #      affTr/cTr split per layer so style(l) streams in) ----
            wpT = wts.tile([128, 4, NB, B], F32, tag="wpT")
            nc.sync.dma_start(wpT[:], wpT_d[:])
            affT0 = wts.tile([128, 4, CIN], F32, tag="affT0")
            nc.sync.dma_start(affT0[:], affT0_d[:])
            ab0 = wts.tile([CIN, 1], F32, tag="ab0")
            nc.sync.dma_start(ab0[:], ab0_d[:])
            abr = wts.tile([HID, NB - 1], F32, tag="abr")
            nc.sync.dma_start(abr[:], abr_d[:])
            cT0 = wts.tile([CIN, HID], F32, tag="cT0")
            nc.sync.dma_start(cT0[:], cT0_d[:])
            gcb0 = wts.tile([HID, 1], F32, tag="gcb0")
            nc.sync.dma_start(gcb0[:], gcb0_d[:])
            gcbr = wts.tile([HID, NB - 1], F32, tag="gcbr")
            nc.sync.dma_start(gcbr[:], gcbr_d[:])
            epsb = wts.tile([HID, 1], F32, tag="epsb")
            nc.vector.memset(epsb[:], EPS * 0.5)
            cTr = wts.tile([HID, NB - 1, HID], F32, tag="cTr")
            affTr = wts.tile([128, 4, NB - 1, HID], F32, tag="affTr")
            for _l in range(NB - 1):
                nc.sync.dma_start(cTr[:, _l, :], cTr_d[:, _l, :])
                nc.sync.dma_start(affTr[:, :, _l, :], affTr_d[:, :, _l, :])

            # ---- style prep for all layers (tiny, runs up-front) ----
            def style(l):
                C = CIN if l == 0 else HID
                affT = (lambda j: affT0[:, j, :]) if l == 0 else (
                    lambda j: affTr[:, j, l - 1, :])
                ab = ab0[:, 0:1] if l == 0 else abr[:, l - 1:l]
                cT = cT0[:] if l == 0 else cTr[:, l - 1, :]

                ps_s = ps.tile([C, B], F32, tag="ps")
                for j in range(4):
                    nc.tensor.matmul(ps_s[:], affT(j), wpT[:, j, l, :],
                                     start=(j == 0), stop=(j == 3))
                sT = sty.tile([C, B], F32, tag="sT")
                nc.scalar.activation(sT[:], ps_s[:],
                                     mybir.ActivationFunctionType.Identity,
                                     bias=ab, scale=INV_SQRT_WDIM)
                ssq = sty.tile([C, B], F32, tag="ssq")
                nc.scalar.activation(ssq[:], sT[:],
                                     mybir.ActivationFunctionType.Square)
                csq = sty.tile([C, HID], F32, tag="csq")
                nc.scalar.activation(csq[:], cT,
                                     mybir.ActivationFunctionType.Square)
                ps_d = ps.tile([HID, B], F32, tag="ps")
                nc.tensor.matmul(ps_d[:], csq[:], ssq[:], start=True, stop=True)
                droot = sty.tile([HID, B], F32, tag="droot")
                nc.scalar.activation(droot[:], ps_d[:],
                                     mybir.ActivationFunctionType.Sqrt,
                                     bias=epsb[:, 0:1], scale=0.5)
                dscale = dscp.tile([HID, B], F32, tag="dscale")
                nc.vector.reciprocal(dscale[:], droot[:])
                wmod = wmodp.tile([C, B * HID], F32R, tag="wmod")
                for b in range(B):
                    nc.vector.tensor_scalar_mul(
                        wmod[:, b * HID:(b + 1) * HID], cT, sT[:, b:b + 1])
                return wmod, dscale

            def iteration(it):
                styles = [style(l) for l in range(NB)]
                gcnt = 0

                def load_block(blk):
                    b, col0, ncols = blk
                    bufA = big.tile([128, ncols], F32R, tag="xbuf")
                    bufB = big.tile([128, ncols], F32R, tag="xbuf")
                    nc.sync.dma_start(bufA[:CIN, :],
                                      x_d[b, :, col0:col0 + ncols])
                    return bufA, bufB

                # column-blocked: load -> 8 layers -> store, prefetch next.
                # First/last batches split in half for shallower ramp/tail.
                blocks = [(b, 0, SHARD) for b in range(B)]
                nxt = load_block(blocks[0])
                for bi, (b, col0, ncols) in enumerate(blocks):
                    bufA, bufB = nxt
                    if bi + 1 < len(blocks):
                        nxt = load_block(blocks[bi + 1])
                    for l in range(NB):
                        C = CIN if l == 0 else HID
                        gcb = gcb0[:, 0:1] if l == 0 else gcbr[:, l - 1:l]
                        x_in = bufA if l % 2 == 0 else bufB
                        x_out = bufB if l % 2 == 0 else bufA
                        last = l == NB - 1
                        wmod, dscale = styles[l]
                        for g in range(ncols // GROUP):
                            pt = ps.tile([128, GROUP], F32, tag="ps")
                            c0 = g * GROUP
                            for t in range(NT):
                                nc.tensor.matmul(
                                    pt[:, t * 512:(t + 1) * 512],
                                    wmod[:C, b * HID:(b + 1) * HID],
                                    x_in[:C, c0 + t * 512:c0 + (t + 1) * 512],
                                    start=True, stop=True)
                            # epilogue: out = prelu(psum*dscale + gcb, 0.2)
                            if last:
                                ost = xst.tile([128, GROUP], F32, tag="xout")
                                o_full = ost[:]
                            else:
                                o_full = x_out[:, c0:c0 + GROUP]
                            if EPI_MODE == "none":
                                nc.scalar.activation(
                                    o_full[:, :4] if last
                                    else x_out[:, c0:c0 + 4],
                                    pt[:, :4],
                                    mybir.ActivationFunctionType.Prelu,
                                    bias=gcb, scale=dscale[:, b:b + 1],
                                    alpha=0.2)
                            else:
                                gi = 0 if (gcnt * ACT_SHARE) % ACT_DEN < ACT_SHARE else 1
                                gcnt += 1
                                if EPI_MODE == "act":
                                    gi = 0
                                elif EPI_MODE == "dve":
                                    gi = 1
                                if gi == 0:
                                    nc.scalar.activation(
                                        o_full, pt[:],
                                        mybir.ActivationFunctionType.Prelu,
                                        bias=gcb, scale=dscale[:, b:b + 1],
                                        alpha=0.2)
                                else:
                                    nc.vector._custom_dve(
                                        SCALE_BIAS_LRELU,
                                        out=o_full, in0=pt[:],
                                        s0=dscale[:, b:b + 1], s1=gcb,
                                        imm2=0.2)
                            if last:
                                nc.gpsimd.dma_start(
                                    y_d[b, :, col0 + g * GROUP:
                                        col0 + (g + 1) * GROUP],
                                    ost[:])

            for it in range(K):
                iteration(it)

    nc.compile()
    return nc


def _prep_inputs(pre_point_features, points_encoding, wp,
                 aff_w_in, aff_b_in, conv_w_in, conv_b_in,
                 aff_w, aff_b, conv_w, conv_b):
    """Host-side layout prep (transposes/reshapes of small parameter tensors)."""
    x = np.ascontiguousarray(np.asarray(points_encoding, np.float32)
                             .reshape(B, CIN, HWTOT))
    wp = np.asarray(wp, np.float32)
    # wpT[p, j, l, b] = wp[b, l, j*128+p]
    wpT = np.ascontiguousarray(
        wp.transpose(2, 1, 0).reshape(4, 128, NB, B).transpose(1, 0, 2, 3))
    aff_w_in = np.asarray(aff_w_in, np.float32)
    affT0 = np.ascontiguousarray(
        aff_w_in.T.reshape(4, 128, CIN).transpose(1, 0, 2))
    aff_w = np.asarray(aff_w, np.float32)
    affTr = np.ascontiguousarray(
        aff_w.transpose(2, 0, 1).reshape(4, 128, NB - 1, HID).transpose(1, 0, 2, 3))
    ab0 = np.ascontiguousarray(np.asarray(aff_b_in, np.float32).reshape(CIN, 1))
    abr = np.ascontiguousarray(np.asarray(aff_b, np.float32).T)
    cT0 = np.ascontiguousarray(np.asarray(conv_w_in, np.float32).T)
    cTr = np.ascontiguousarray(np.asarray(conv_w, np.float32).transpose(2, 0, 1))
    gcb0 = np.ascontiguousarray(
        (SQRT2 * np.asarray(conv_b_in, np.float32)).reshape(HID, 1))
    gcbr = np.ascontiguousarray(SQRT2 * np.asarray(conv_b, np.float32).T)

    shared = dict(wpT=wpT, affT0=affT0, affTr=affTr, ab0=ab0, abr=abr,
                  cT0=cT0, cTr=cTr, gcb0=gcb0, gcbr=gcbr)
    in_maps = []
    for c in range(N_CORES):
        m = dict(shared)
        m["x"] = np.ascontiguousarray(x[:, :, c * SHARD:(c + 1) * SHARD])
        in_maps.append(m)
    return in_maps


def kernel(trace=False, **inputs):
    global _COMPILED
    if _COMPILED is None:
        _COMPILED = _build()
    nc = _COMPILED
    in_maps = _prep_inputs(**inputs)
    res = run_bass_kernel_spmd(nc, in_maps, core_ids=list(range(N_CORES)),
                               trace=trace)
    parts = [res.results[c]["y"] for c in range(N_CORES)]
    out = np.concatenate(parts, axis=2).reshape(B, HID, H, W)
    if trace:
        kernel.last_result = res
    return out

